# revision 5
# baseline (speedup 1.0000x reference)
"""Deformable Conv1D on 8 Trainium2 NeuronCores (Bass/Tile).

Math (reference): out[b,o,l] = sum_{i,k} W[o,i,k] * interp[b,i,l,k] + bias[o]
  interp[b,i,l,k] = wa*x[b,i,x0c] + wb*x[b,i,x1c],  loc = l + k + off[b,l,k]
  x0c/x1c = clip(floor(loc))/clip(floor(loc)+1), wa = x1c-loc, wb = loc-x0c.

Device decomposition per core (core j: batch b=j//2, L-half S=4096*(j%2)):
  Phase 1 (PE): Y_k^T[t, o] = sum_i x[b,i,t] * W[o,i,k]   (matmul, fp32r)
  Phase 2 (PE): out^T[l, o] = sum_k sum_t G_k[t, l] * Y_k^T[t, o]
    where G_k is a host-built banded selector holding the interpolation
    weights wa/wb at rows t = x0c/x1c (offsets are data-dependent but small:
    |floor(off)| <= 4, so a 128-row band covers a 113-wide output chunk).
  Host does: the tiny offset conv (2.7% of FLOPs), G assembly (pure
  addressing), and the final [l,o] -> [o,l] transpose.

All heavy FLOPs (30.1 GFLOP of matmul) run on the PE engines of 8 cores.
"""

import numpy as np

import concourse.bacc as bacc
import concourse.bass as bass
import concourse.mybir as mybir
import concourse.tile as tile
from concourse.bass_utils import run_bass_kernel_spmd

# Problem constants (hardcoded per harness contract).
B, CIN, COUT, L = 4, 256, 256, 8192
K, PAD = 7, 3
NCORE = 8
HALF = L // 2              # 4096 output positions per core
CHUNK = 113                # output positions per window (band 128 covers s in [-4,4])
NWIN = -(-HALF // CHUNK)   # 37
XPW = 4224                 # padded x width per core (needs 113*36+128 = 4196)
HALO = 4                   # x_pad global col 0 == S - HALO
F32 = mybir.dt.float32
F32R = mybir.dt.float32r

_NC_CACHE = {}


def _build_nc():
    if "nc" in _NC_CACHE:
        return _NC_CACHE["nc"]
    nc = bacc.Bacc("TRN2", target_bir_lowering=False, debug=False, num_devices=NCORE)
    x_d = nc.dram_tensor("xp", [2, 128, XPW], F32R, kind="ExternalInput")
    w_d = nc.dram_tensor("wt", [2, K, 128, COUT], F32R, kind="ExternalInput")
    g_d = nc.dram_tensor("gsel", [NWIN, K, 128, CHUNK], F32R, kind="ExternalInput")
    b_d = nc.dram_tensor("bias", [1, COUT], F32, kind="ExternalInput")
    o_d = nc.dram_tensor("out", [HALF, COUT], F32, kind="ExternalOutput")

    with tile.TileContext(nc) as tc:
        with (
            tc.tile_pool(name="const", bufs=1) as cpool,
            tc.tile_pool(name="gp", bufs=3) as gpool,
            tc.tile_pool(name="yp", bufs=2) as ypool,
            tc.tile_pool(name="op", bufs=3) as opool,
            tc.tile_pool(name="ps1", bufs=7, space="PSUM") as ps1,
            tc.tile_pool(name="ps2", bufs=1, space="PSUM") as ps2,
        ):
            # ---- constants: x halves, weights, bias tile ----
            x_sb = []
            for i in range(2):
                xt = cpool.tile([128, XPW], F32R, tag=f"x{i}")
                nc.sync.dma_start(xt[:], x_d[i])
                x_sb.append(xt)
            w_sb = cpool.tile([128, 2, K, COUT], F32R, tag="w")
            nc.sync.dma_start(w_sb[:], w_d.rearrange("i k p o -> p i k o"))
            bias_row = cpool.tile([1, COUT], F32, tag="br")
            nc.sync.dma_start(bias_row[:], b_d[:])
            ones_col = cpool.tile([1, CHUNK], F32, tag="oc")
            nc.vector.memset(ones_col[:], 1.0)
            bias_ps = ps2.tile([CHUNK, COUT], F32, tag="ops")
            nc.tensor.matmul(bias_ps[:], ones_col[:], bias_row[:], start=True, stop=True)
            bias_tile = cpool.tile([CHUNK, COUT], F32, tag="bt")
            nc.vector.tensor_copy(bias_tile[:], bias_ps[:])

            # ---- software-pipelined window loop ----
            state = {}  # window ci -> (g_tile, [y_k tiles])

            def phase1(ci):
                gt = gpool.tile([128, K, CHUNK], F32R, tag="g")
                nc.sync.dma_start(gt[:], g_d[ci].rearrange("k p q -> p k q"))
                yps = [ps1.tile([128, COUT], F32, tag="yps", name=f"yps{k}")
                       for k in range(K)]
                for i in range(2):
                    lhs = x_sb[i][:, CHUNK * ci:CHUNK * ci + 128]
                    for k in range(K):
                        nc.tensor.matmul(yps[k][:], lhs, w_sb[:, i, k, :],
                                         start=(i == 0), stop=(i == 1))
                ys = []
                for k in range(K):
                    yt = ypool.tile([128, COUT], F32R, tag=f"y{k}", name=f"y{k}")
                    eng = nc.vector if k % 2 == 0 else nc.scalar
                    if eng is nc.vector:
                        nc.vector.tensor_copy(yt[:], yps[k][:])
                    else:
                        nc.scalar.copy(yt[:], yps[k][:])
                    ys.append(yt)
                state[ci] = (gt, ys)

            def phase2(ci):
                gt, ys = state.pop(ci)
                ops = ps2.tile([CHUNK, COUT], F32, tag="ops")
                for k in range(K):
                    nc.tensor.matmul(ops[:], gt[:, k, :], ys[k][:],
                                     start=(k == 0), stop=(k == K - 1))
                osb = opool.tile([CHUNK, COUT], F32, tag="o")
                nc.vector.tensor_add(osb[:], ops[:], bias_tile[:])
                rows = min(CHUNK, HALF - CHUNK * ci)
                nc.sync.dma_start(o_d[CHUNK * ci:CHUNK * ci + rows, :], osb[:rows, :])

            for ci in range(NWIN):
                phase1(ci)
                if ci > 0:
                    phase2(ci - 1)
            phase2(NWIN - 1)

    nc.finalize()
    _NC_CACHE["nc"] = nc
    return nc


def _host_prep(x, weight, bias, offset_w, offset_b):
    """Offset conv + selector build on host. Returns per-core input maps."""
    x = np.ascontiguousarray(x, np.float32)
    weight = np.asarray(weight, np.float32)
    bias = np.asarray(bias, np.float32)
    offset_w = np.asarray(offset_w, np.float32)
    offset_b = np.asarray(offset_b, np.float32)

    # offsets[b, kk, l] (same math as reference conv, fp32)
    xpc = np.zeros((B, CIN, L + 2 * PAD), np.float32)
    xpc[:, :, PAD:PAD + L] = x
    offs = np.zeros((B, K, L), np.float32)
    for k2 in range(K):
        offs += np.einsum("kc,bcl->bkl", offset_w[:, :, k2],
                          xpc[:, :, k2:k2 + L], optimize=True)
    offs += offset_b[None, :, None]

    # loc per (b, l, k); p + p_k + PAD == l + k exactly in fp32
    lk = (np.arange(L, dtype=np.float32)[:, None]
          + np.arange(K, dtype=np.float32)[None, :])      # [L, K]
    loc = lk[None] + np.transpose(offs, (0, 2, 1))        # [B, L, K]
    x0 = np.floor(loc).astype(np.int64)
    x0c = np.clip(x0, 0, L - 1)
    x1c = np.clip(x0 + 1, 0, L - 1)
    wa = x1c.astype(np.float32) - loc
    wb = loc - x0c.astype(np.float32)

    wt = np.ascontiguousarray(
        weight.reshape(COUT, 2, 128, K).transpose(1, 3, 2, 0))  # [i,k,p,o]
    bias_row = bias.reshape(1, COUT)

    in_maps = []
    for core in range(NCORE):
        b, half = divmod(core, 2)
        S = HALF * half
        # x_pad: global cols [S-HALO, S-HALO+XPW)
        xp = np.zeros((CIN, XPW), np.float32)
        lo, hi = S - HALO, S - HALO + XPW
        cl, ch = max(0, lo), min(L, hi)
        xp[:, cl - lo:ch - lo] = x[b, :, cl:ch]

        # selector G[ci, k, u, q]
        G = np.zeros((NWIN, K, 128, CHUNK), np.float32)
        l_idx = S + np.arange(HALF)                      # global l for q-slots
        ci = np.arange(HALF) // CHUNK
        q = np.arange(HALF) % CHUNK
        band0 = (S + ci * CHUNK - HALO)                  # global band start
        for k in range(K):
            u0 = x0c[b, l_idx, k] - band0
            u1 = x1c[b, l_idx, k] - band0
            if u0.min() < 0 or u1.max() > 127:
                raise AssertionError(
                    f"offset out of band: u0min={u0.min()} u1max={u1.max()}")
            flat = G.reshape(-1)
            base = ((ci * K + k) * 128)
            np.add.at(flat, (base + u0) * CHUNK + q, wa[b, l_idx, k])
            np.add.at(flat, (base + u1) * CHUNK + q, wb[b, l_idx, k])

        in_maps.append({
            "xp": np.ascontiguousarray(xp.reshape(2, 128, XPW)),
            "wt": wt,
            "gsel": G,
            "bias": bias_row,
        })
    return in_maps


def _assemble(results):
    out = np.empty((B, COUT, L), np.float32)
    for b in range(B):
        lo_half = results[2 * b]["out"]       # [4096, 256] rows l in [0,4096)
        hi_half = results[2 * b + 1]["out"]
        out[b, :, :HALF] = lo_half.T
        out[b, :, HALF:] = hi_half.T
    return out


def kernel(x, weight, bias, offset_w, offset_b):
    nc = _build_nc()
    in_maps = _host_prep(x, weight, bias, offset_w, offset_b)
    res = run_bass_kernel_spmd(nc, in_maps, core_ids=list(range(NCORE)))
    return _assemble(res.results)


def kernel_timed(inputs, repeats=3):
    """Dev helper: returns (out, wall_times_s per run)."""
    import time
    nc = _build_nc()
    in_maps = _host_prep(**inputs)
    times, res = [], None
    for _ in range(repeats):
        t0 = time.time()
        res = run_bass_kernel_spmd(nc, in_maps, core_ids=list(range(NCORE)))
        times.append(time.time() - t0)
    return _assemble(res.results), times
